# revision 48
# baseline (speedup 1.0000x reference)
"""Trainium2 Bass kernel for nn_EpipolarWarpOperator (B=8, C=320, H=W=64, S=3).

Sharding: spatial — every core computes an 8-row horizontal strip of all 8
batch outputs (the work of a batch is dominated by the 3x3 conv, which is
strip-separable; the epipolar sampling is handled per strip with halo).

Host analysis classifies each batch by its epipolar sampling map:

* pattern batches: the per-pixel bilinear sampling signature map has few
  distinct values (the epipolar lines clip to a handful of source locations),
  so the sampled image is piecewise constant and each output column is one of
  npat distinct "pattern" columns (3x3 signature neighborhoods incl. the
  zero-pad border). Per batch the device computes R[k, tap, pi] (replicated
  sampled columns) from a host-gathered slab via tiny matmuls,
  T^T[pi, m] = sum_{tap,kc} R_tap.T @ W_tap (+bias via a ones-row), relu on
  ACT, then expands out = T^T.T @ E with a per-core 0/1 selection matrix E.
  All x-dependent math stays on device.

* dense batches: per (128-pixel block, sample) the host pre-gathers the
  <=128 distinct bilinear corner pixels into a slab [128, 320] plus a
  sampling matrix S [128, 128]; the device runs swapped-operand matmuls
  slab.T @ S accumulating all samples (and sub-slabs) into PSUM, directly in
  channel-major order, then a 27-matmul 3x3 conv over the strip (+halo row)
  with bias+relu on ACT.
"""

import numpy as np

B, C, H, W = 8, 320, 64, 64
SN = 3
HW = H * W
STRIP = 8             # output rows per core
NCORE = 8
MB = [(0, 128), (128, 128), (256, 64)]   # output/input channel chunking
PI_MAX = 126          # total pattern budget (K of the expansion matmul)
NPAT_MAX = 56         # per-batch pattern cap (9*npat*4B must fit a PSUM bank)
NSIG_MAX = 64
NBLK = 6              # 2-row sampling blocks per strip (incl. 2-row halo pad)

# ---------------------------------------------------------------- host: geometry


def _rodrigues_np(rv):
    theta = np.sqrt((rv * rv).sum())
    r = rv / max(theta, 1e-12)
    I = np.eye(3, dtype=np.float32)
    K = np.array([[0, -r[2], r[1]], [r[2], 0, -r[0]], [-r[1], r[0], 0]],
                 dtype=np.float32)
    R = np.cos(theta) * I + (1 - np.cos(theta)) * np.outer(r, r) + np.sin(theta) * K
    return I if theta < 1e-6 else R


def fundamental_np(Ks, Kt, ps, pt):
    Fs = []
    for b in range(Ks.shape[0]):
        Rs = _rodrigues_np(ps[b, :3].astype(np.float32))
        Rt = _rodrigues_np(pt[b, :3].astype(np.float32))
        ts_, tt_ = ps[b, 3:].astype(np.float32), pt[b, 3:].astype(np.float32)
        R_rel = Rs @ Rt.T
        t_rel = ts_ - R_rel @ tt_
        z = np.float32(0)
        skew = np.array([[z, -t_rel[2], t_rel[1]],
                         [t_rel[2], z, -t_rel[0]],
                         [-t_rel[1], t_rel[0], z]], dtype=np.float32)
        E = skew @ R_rel
        inv_Ks = np.linalg.inv(Ks[b].astype(np.float32))
        inv_Kt = np.linalg.inv(Kt[b].astype(np.float32))
        Fs.append(inv_Kt.T @ E @ inv_Ks)
    return np.stack(Fs).astype(np.float32)


def geometry(F):
    k = np.arange(HW)
    px = (k % W).astype(np.float32)
    py = (k // W).astype(np.float32)
    P = np.stack([px, py, np.ones_like(px)])
    lines = F.T.astype(np.float32) @ P
    a, b_, c = lines[0], lines[1], lines[2]
    W1, H1 = np.float32(W - 1), np.float32(H - 1)
    EPS = np.float32(1e-10)
    x1 = np.clip(-c / (a + EPS), 0.0, W1)
    x2 = np.clip(-(b_ * H1 + c) / (a + EPS), 0.0, W1)
    y1 = np.clip(-c / (b_ + EPS), 0.0, H1)
    y2 = np.clip(-(a * W1 + c) / (b_ + EPS), 0.0, H1)
    t = np.linspace(0.0, 1.0, SN, dtype=np.float32)
    sx = x1[:, None] * (1 - t) + x2[:, None] * t
    sy = y1[:, None] * (1 - t) + y2[:, None] * t
    x0 = np.floor(sx)
    y0 = np.floor(sy)
    wx = (sx - x0).astype(np.float32)
    wy = (sy - y0).astype(np.float32)
    x0i = np.clip(x0, 0, W - 1).astype(np.int32)
    y0i = np.clip(y0, 0, H - 1).astype(np.int32)
    return x0i, y0i, wx, wy


def _corners(geo, p, s):
    """4 bilinear (source pixel row, weight/3) pairs for pixel p, sample s."""
    x0i, y0i, wx, wy = geo
    y0 = int(y0i[p, s]); x0 = int(x0i[p, s])
    x1 = min(x0 + 1, W - 1); y1 = min(y0 + 1, H - 1)
    wxx = np.float32(wx[p, s]); wyy = np.float32(wy[p, s])
    third = np.float32(1.0 / 3.0)
    out = {}
    for ry, rx, ww in ((y0, x0, (1 - wxx) * (1 - wyy)),
                       (y0, x1, wxx * (1 - wyy)),
                       (y1, x0, (1 - wxx) * wyy),
                       (y1, x1, wxx * wyy)):
        rr = ry * W + rx
        out[rr] = out.get(rr, np.float32(0)) + ww * third
    return out


# ------------------------------------------------------------- host: classify


def classify(geo):
    """If the sampling map has <= NSIG_MAX distinct per-pixel signatures,
    return (siginv [HW], tables); else None."""
    x0i, y0i, wx, wy = geo
    key = np.concatenate([
        y0i.astype(np.float32), x0i.astype(np.float32), wx, wy], axis=1)
    kview = np.ascontiguousarray(key).view([('', key.dtype)] * key.shape[1]).ravel()
    uniq, first, inv = np.unique(kview, return_index=True, return_inverse=True)
    if len(uniq) > NSIG_MAX:
        return None
    tables = []
    for si in range(len(uniq)):
        p = int(first[si])
        contrib = {}
        for s in range(SN):
            for rr, ww in _corners(geo, p, s).items():
                contrib[rr] = contrib.get(rr, np.float32(0)) + ww
        tables.append(sorted(contrib.items()))
    return inv.astype(np.int32), tables


def patterns(siginv):
    """3x3 neighborhood patterns of the signature map (border = -1)."""
    simg = siginv.reshape(H, W)
    pad = np.pad(simg, 1, constant_values=-1)
    neigh = np.stack([pad[dy:dy + H, dx:dx + W].ravel()
                      for dy in range(3) for dx in range(3)], axis=1)
    uniq, pinv = np.unique(neigh, axis=0, return_inverse=True)
    return pinv.astype(np.int32), uniq


# ------------------------------------------------------------ host: build plan


def _deg_slabs(entries):
    """Assign signatures (from possibly several batches) to slabs so each
    slab's source-pixel union <= 128. entries: [(bloc, si, table)] with
    table = [(pixrow, w)]. Returns list of (rowmap, sigs): rowmap =
    {(bloc, pixrow): slabrow}, sigs = [(bloc, si)]."""
    slabs = []
    rows, sigs = {}, []
    for bloc, si, tab in entries:
        new = [(bloc, rr) for rr, _ in tab if (bloc, rr) not in rows]
        if len(rows) + len(new) > 128:
            slabs.append((rows, sigs))
            rows, sigs = {}, []
            new = [(bloc, rr) for rr, _ in tab]
        for key in new:
            rows[key] = len(rows)
        sigs.append((bloc, si))
    slabs.append((rows, sigs))
    return slabs


def _dense_block_slabs(pix_ids, geo):
    """Greedy split of a 128-pixel block's (pixel, sample) tokens into slabs
    of <=128 distinct corner pixels, tokens sorted by corner location so
    nearby tokens share slab rows. Returns list of (rowmap, S[128,128])."""
    x0i, y0i = geo[0], geo[1]
    toks = [(int(y0i[p, s]), int(x0i[p, s]), i, p, s)
            for i, p in enumerate(pix_ids) if p >= 0 for s in range(SN)]
    toks.sort()
    slabs = []
    rows = {}
    S = np.zeros((128, 128), dtype=np.float32)
    for _, _, i, p, s in toks:
        cs = _corners(geo, p, s)
        new = [rr for rr in cs if rr not in rows]
        if len(rows) + len(new) > 128:
            slabs.append((rows, S))
            rows = {}
            S = np.zeros((128, 128), dtype=np.float32)
            new = list(cs)
        for rr in new:
            rows[rr] = len(rows)
        for rr, ww in cs.items():
            S[rows[rr], i] += ww
    if rows or not slabs:
        slabs.append((rows, S))
    return slabs


def make_plan(x, source_intrinsics, target_intrinsics, source_pose,
              target_pose, conv_w, conv_b):
    Fs = fundamental_np(np.asarray(source_intrinsics, np.float32),
                        np.asarray(target_intrinsics, np.float32),
                        np.asarray(source_pose, np.float32),
                        np.asarray(target_pose, np.float32))
    x = np.asarray(x, np.float32)
    xT16 = [np.ascontiguousarray(x[b].reshape(C, HW).T).astype(np.float16)
            for b in range(B)]

    degs, denses = [], []
    pi_used = 0
    for b in range(B):
        geo = geometry(Fs[b])
        res = classify(geo)
        if res is not None:
            siginv, tables = res
            pinv, ptab = patterns(siginv)
            npat = ptab.shape[0]
            if npat <= NPAT_MAX and pi_used + npat <= PI_MAX:
                degs.append(dict(gb=b, siginv=siginv, tables=tables,
                                 pinv=pinv, ptab=ptab, pi_off=pi_used))
                pi_used += npat
                continue
        denses.append(dict(gb=b, geo=geo))
    pi_tot = pi_used

    # ---- degenerate global data (same for all cores) ----
    # pack consecutive degenerate batches into groups with sum 9*npat <= 512
    # (one PSUM bank); each group's signatures are slab-packed jointly so
    # tiny batches share slabs, with column layout pi-major per batch
    import os as _os
    RGROUP_CAP = int(_os.environ.get("K_RGCAP", "512"))
    rgroups = []          # dicts: bis (deg indices), ncols, slabs
    cur, cur_n = [], 0
    for bi, d in enumerate(degs):
        n9 = 9 * d['ptab'].shape[0]
        if cur and cur_n + n9 > RGROUP_CAP:
            rgroups.append(dict(bis=cur, ncols=cur_n))
            cur, cur_n = [], 0
        cur.append(bi)
        cur_n += n9
    if cur:
        rgroups.append(dict(bis=cur, ncols=cur_n))

    sg_blocks, sr_blocks = [], []
    for g in rgroups:
        entries = []
        for bloc, bi in enumerate(g['bis']):
            for si, tab in enumerate(degs[bi]['tables']):
                entries.append((bloc, si, tab))
        slabs = _deg_slabs(entries)
        g['nslab'] = len(slabs)
        g['q0'] = degs[g['bis'][0]]['pi_off']
        g['nq'] = sum(degs[bi]['ptab'].shape[0] for bi in g['bis'])
        # group columns are tap-major (col = tap*nq + qlocal) so the PSUM
        # drain into rsb[.., kc, tap, q] is a plain contiguous-inner copy
        nq = g['nq']
        for rows, sigs in slabs:
            slab = np.zeros((128, C), dtype=np.float16)
            for (bloc, rr), idx in rows.items():
                slab[idx] = xT16[degs[g['bis'][bloc]]['gb']][rr]
            SR = np.zeros((128, g['ncols']), dtype=np.float32)
            sigset = set(sigs)
            for bloc, bi in enumerate(g['bis']):
                d = degs[bi]
                ptab = d['ptab']
                qb = d['pi_off'] - g['q0']
                for pi in range(ptab.shape[0]):
                    for tap in range(9):
                        si = ptab[pi, tap]
                        if si >= 0 and (bloc, si) in sigset:
                            for rr, ww in d['tables'][si]:
                                SR[rows[(bloc, rr)],
                                   tap * nq + qb + pi] += ww
            sg_blocks.append(slab)
            sr_blocks.append(SR.astype(np.float16))
    sg = (np.concatenate(sg_blocks, axis=1) if sg_blocks
          else np.zeros((128, 0), np.float16))
    sr = (np.concatenate(sr_blocks, axis=1) if sr_blocks
          else np.zeros((128, 0), np.float16))

    # ---- dense per-core blocks ----
    # block = 2 image rows 8r-2+2*blk, +1 of a dense batch (halo included;
    # out-of-image rows get zero columns); all 3 samples share its slabs
    core_units = []   # [core][block] -> list of (rowmap, S)
    for r in range(NCORE):
        units = []
        for d in denses:
            for blk in range(NBLK):
                row0 = 8 * r - 2 + 2 * blk
                pix = []
                for lr in range(2):
                    row = row0 + lr
                    pix += [row * W + cx if 0 <= row < H else -1
                            for cx in range(W)]
                units.append(_dense_block_slabs(pix, d['geo']))
        core_units.append(units)
    nunits = len(core_units[0])
    unit_nslab = [max(1, max(len(core_units[r][u]) for r in range(NCORE)))
                  for u in range(nunits)]

    ui = 0
    for d in denses:
        d['unit_nslab'] = tuple(unit_nslab[ui:ui + NBLK])
        ui += NBLK

    # ---- weights ----
    Wl = np.zeros((128, 3 * 9 * C), dtype=np.float16)
    cw = np.asarray(conv_w, np.float32)
    cb = np.asarray(conv_b, np.float32)
    for kc, (koff, ksz) in enumerate(MB):
        for tap in range(9):
            dy, dx = tap // 3, tap % 3
            Wl[0:ksz, kc * 9 * C + tap * C: kc * 9 * C + tap * C + C] = \
                cw[:, koff:koff + ksz, dy, dx].T.astype(np.float16)
    # paired kc=2 weights for the dense conv: partitions 0:64 hold the
    # dy=0 tap, 64:128 the dy=1 tap (read through the +1-row shifted
    # duplicate of the kc=2 sampled plane)
    Wl2 = np.zeros((128, 3 * C), dtype=np.float16)
    for dxi in range(3):
        Wl2[0:64, dxi * C: dxi * C + C] = \
            cw[:, 256:320, 0, dxi].T.astype(np.float16)
        Wl2[64:128, dxi * C: dxi * C + C] = \
            cw[:, 256:320, 1, dxi].T.astype(np.float16)
    bias = np.zeros((128, 3), dtype=np.float32)
    for mc, (moff, msz) in enumerate(MB):
        bias[0:msz, mc] = cb[moff:moff + msz]

    # ---- per-core in_maps ----
    ndeg, ndense = len(degs), len(denses)
    slots = [d['gb'] for d in degs] + [d['gb'] for d in denses]
    in_maps = []
    for r in range(NCORE):
        m = {"wl": Wl, "wl2": Wl2, "bias": bias}
        if ndeg:
            m["sg"] = sg
            m["sr"] = sr
            e = np.zeros((128, ndeg * STRIP * W), dtype=np.float16)
            for bi, d in enumerate(degs):
                pidx = d['pinv'].reshape(H, W)[8 * r: 8 * r + STRIP].ravel()
                e[d['pi_off'] + pidx,
                  bi * STRIP * W + np.arange(STRIP * W)] = 1.0
            m["e_mat"] = e
        if ndense:
            sds, sss = [], []
            for u in range(nunits):
                slabs = core_units[r][u]
                di = u // NBLK
                gb = denses[di]['gb']
                for j in range(unit_nslab[u]):
                    slab = np.zeros((128, C), dtype=np.float16)
                    S = np.zeros((128, 128), dtype=np.float16)
                    if j < len(slabs):
                        rows, Sf = slabs[j]
                        rl = sorted(rows, key=rows.get)
                        if rl:
                            slab[:len(rl)] = xT16[gb][np.array(rl)]
                        S = Sf.astype(np.float16)
                    sds.append(slab)
                    sss.append(S)
            m["sd"] = np.concatenate(sds, axis=1)
            m["ss"] = np.concatenate(sss, axis=1)
        in_maps.append(m)

    struct = (pi_tot,
              tuple((d['gb'], d['ptab'].shape[0]) for d in degs),
              tuple((g['ncols'], g['nslab'], g['q0'], g['nq'])
                    for g in rgroups),
              tuple((d['gb'], d['unit_nslab']) for d in denses))
    return in_maps, struct, slots


# ------------------------------------------------------------- bass program

_NC_CACHE = {}


def build_program(reps, struct):
    key = (reps, struct)
    if key in _NC_CACHE:
        return _NC_CACHE[key]
    import concourse.bacc as bacc
    import concourse.mybir as mybir
    from concourse.tile import TileContext

    fp16 = mybir.dt.float16
    f32 = mybir.dt.float32
    pi_tot, degs, rgroups, denses = struct
    ndeg, ndense = len(degs), len(denses)
    NB = ndeg + ndense
    NSG = sum(ns for _, ns, _, _ in rgroups)
    SRC = sum(nc_ * ns for nc_, ns, _, _ in rgroups)
    NSLAB = sum(sum(us) for _, us in denses)
    SW = STRIP * W   # 512 pixels per strip

    nc = bacc.Bacc(target_bir_lowering=False)
    wl_d = nc.dram_tensor("wl", [128, 3 * 9 * C], fp16, kind="ExternalInput")
    wl2_d = nc.dram_tensor("wl2", [128, 3 * C], fp16, kind="ExternalInput")
    bias_d = nc.dram_tensor("bias", [128, 3], f32, kind="ExternalInput")
    if ndeg:
        sg_d = nc.dram_tensor("sg", [128, NSG * C], fp16, kind="ExternalInput")
        sr_d = nc.dram_tensor("sr", [128, SRC], fp16, kind="ExternalInput")
        e_d = nc.dram_tensor("e_mat", [128, ndeg * SW], fp16,
                             kind="ExternalInput")
    if ndense:
        sd_d = nc.dram_tensor("sd", [128, NSLAB * C], fp16,
                              kind="ExternalInput")
        ss_d = nc.dram_tensor("ss", [128, NSLAB * 128], fp16,
                              kind="ExternalInput")
    out_d = nc.dram_tensor("out", [128, NB * 3 * SW], fp16,
                           kind="ExternalOutput")

    with TileContext(nc) as tc:
        with tc.tile_pool(name="const", bufs=1) as constp, \
             tc.tile_pool(name="inp", bufs=2) as inp, \
             tc.tile_pool(name="sdp", bufs=2) as sdp, \
             tc.tile_pool(name="ssp", bufs=2) as ssp, \
             tc.tile_pool(name="work", bufs=2) as work, \
             tc.tile_pool(name="smpp", bufs=2) as smpp, \
             tc.tile_pool(name="outp", bufs=2) as outp, \
             tc.tile_pool(name="psA", bufs=3, space="PSUM") as psA, \
             tc.tile_pool(name="psB", bufs=5, space="PSUM") as psB:
            wl = constp.tile([128, 3 * 9 * C], fp16)
            nc.sync.dma_start(out=wl[:], in_=wl_d[:])
            wl2 = constp.tile([128, 3 * C], fp16)
            nc.sync.dma_start(out=wl2[:], in_=wl2_d[:])
            bias_t = constp.tile([128, 3], f32)
            nc.sync.dma_start(out=bias_t[:], in_=bias_d[:])

            def body(_it):
                if True:
                    out_sb = outp.tile([128, NB, 3, SW], fp16, name="out_sb",
                                       tag="out_sb")
                    # mc=2 has only 64 valid channel partitions; zero the rest
                    # so the out DMA never reads uninitialized SBUF
                    nc.gpsimd.memset(out_sb[64:128, :, 2:3, :], 0.0)

                    # ---------- input DMAs ----------
                    if ndeg:
                        sg = inp.tile([128, NSG * C], fp16, name="sg",
                                      tag="sg")
                        nc.sync.dma_start(out=sg[:], in_=sg_d[:])
                        sr = inp.tile([128, SRC], fp16, name="sr", tag="sr")
                        nc.sync.dma_start(out=sr[:], in_=sr_d[:])
                    sd_tiles, ss_tiles = [], []
                    if ndense:
                        # chunk slab streams per (dense batch, block pair)
                        off = 0
                        for di, (_, us) in enumerate(denses):
                            for b2 in range(0, NBLK, 2):
                                nsl = us[b2] + us[b2 + 1]
                                sdt = sdp.tile([128, nsl * C], fp16,
                                               tag=f"sd{b2}")
                                nc.sync.dma_start(
                                    out=sdt[:],
                                    in_=sd_d[:, off * C:(off + nsl) * C])
                                sst = ssp.tile([128, nsl * 128], fp16,
                                               tag=f"ss{b2}")
                                nc.sync.dma_start(
                                    out=sst[:],
                                    in_=ss_d[:, off * 128:(off + nsl) * 128])
                                sd_tiles += [(sdt, 0), (sdt, us[b2])]
                                ss_tiles += [(sst, 0), (sst, us[b2])]
                                off += nsl
                    if ndeg:
                        # needed only by the late expansion matmuls, and only
                        # rows 0:pi_tot — issued after the sampling streams
                        e = inp.tile([128, ndeg * SW], fp16, name="e",
                                     tag="e")
                        nc.sync.dma_start(out=e[0:pi_tot, :],
                                          in_=e_d[0:pi_tot, :])

                    # ---------- degenerate path: R ----------
                    if ndeg:
                        rsb = work.tile([128, 3, 9, pi_tot], fp16, name="rsb",
                                        tag="rsb")
                        sgo, sro = 0, 0
                        for gi, (ncols, nsl, q0, nq) in enumerate(rgroups):
                            for kc, (koff, ksz) in enumerate(MB):
                                ps_r = psA.tile([128, 512], f32,
                                                name=f"psr{gi}_{kc}",
                                                tag="psA")
                                for j in range(nsl):
                                    nc.tensor.matmul(
                                        ps_r[0:ksz, 0:ncols],
                                        sg[:, (sgo + j) * C + koff:
                                           (sgo + j) * C + koff + ksz],
                                        sr[:, sro + j * ncols:
                                           sro + (j + 1) * ncols],
                                        start=(j == 0), stop=(j == nsl - 1))
                                psq = ps_r[:, 0:ncols].rearrange(
                                    "p (t q) -> p t q", t=9)
                                nc.vector.tensor_copy(
                                    rsb[0:ksz, kc, :, q0:q0 + nq],
                                    psq[0:ksz])
                            sgo += nsl
                            sro += nsl * ncols

                    # ---------- dense sampling (first half) ----------
                    smps = []
                    if ndense:
                        for di in range(ndense):
                            smp = smpp.tile([128, 3, 2 * NBLK, 66], fp16,
                                            name=f"smp{di}", tag=f"smp{di}")
                            nc.gpsimd.memset(smp[:, :, :, 0:1], 0.0)
                            nc.gpsimd.memset(smp[:, :, :, 65:66], 0.0)
                            smps.append(smp)

                    def dense_block(di, blk):
                        _, us = denses[di]
                        smp = smps[di]
                        sdt, sdo = sd_tiles[di * NBLK + blk]
                        sst, sso = ss_tiles[di * NBLK + blk]
                        ps = psA.tile([128, 512], f32,
                                      name=f"psb{di}_{blk}", tag="psA")
                        nsl = us[blk]
                        for kc, (koff, ksz) in enumerate(MB):
                            for k in range(nsl):
                                nc.tensor.matmul(
                                    ps[0:ksz, kc * 128:(kc + 1) * 128],
                                    sdt[:, (sdo + k) * C + koff:
                                        (sdo + k) * C + koff + ksz],
                                    sst[:, (sso + k) * 128:
                                        (sso + k + 1) * 128],
                                    start=(k == 0), stop=(k == nsl - 1))
                        psv = ps[:, 0:384].rearrange("p (k r c) -> p k r c",
                                                     k=3, r=2)
                        nc.vector.tensor_copy(
                            smp[0:128, 0:2, 2 * blk:2 * blk + 2, 1:65],
                            psv[0:128, 0:2, :, :])
                        nc.vector.tensor_copy(
                            smp[0:64, 2:3, 2 * blk:2 * blk + 2, 1:65],
                            psv[0:64, 2:3, :, :])
                        if blk >= 1:
                            # +1-row shifted duplicate of the kc=2 plane in
                            # partitions 64:128, for the paired conv taps
                            nc.vector.tensor_copy(
                                smp[64:128, 2:3, 2 * blk - 1:2 * blk + 1,
                                    1:65],
                                psv[0:64, 2:3, :, :])

                    if ndense:
                        for di in range(ndense):
                            for blk in range(3):
                                dense_block(di, blk)

                    # ---------- degenerate path: T ----------
                    # bias + relu commute with the per-pixel column selection,
                    # so they are applied after the expansion matmul instead
                    if ndeg:
                        ps_t = psB.tile([128, 512], f32, name="ps_t",
                                        tag="psB")
                        k = 0
                        for kc, (koff, ksz) in enumerate(MB):
                            for tap in range(9):
                                nc.tensor.matmul(
                                    ps_t[0:pi_tot, 0:C],
                                    rsb[0:ksz, kc, tap, :],
                                    wl[0:ksz, kc * 9 * C + tap * C:
                                       kc * 9 * C + tap * C + C],
                                    start=(k == 0), stop=(k == 26))
                                k += 1
                        tsb = work.tile([128, C], fp16, name="tsb", tag="tsb")
                        nc.scalar.copy(tsb[0:pi_tot, :], ps_t[0:pi_tot, 0:C])

                    # ---------- dense sampling (second half) ----------
                    if ndense:
                        for di in range(ndense):
                            for blk in range(3, NBLK):
                                dense_block(di, blk)

                    # ---------- expansion + dense conv, interleaved per mc ----
                    # the conv matmul groups run on PE while the expansion
                    # PSUM drains complete on DVE/ACT
                    for mc, (moff, msz) in enumerate(MB):
                        if ndeg:
                            for bi in range(ndeg):
                                ps_e = psB.tile([128, 512], f32,
                                                name=f"pse{mc}_{bi}",
                                                tag="psB")
                                nc.tensor.matmul(
                                    ps_e[0:msz, :],
                                    tsb[0:pi_tot, moff:moff + msz],
                                    e[0:pi_tot, bi * SW:(bi + 1) * SW],
                                    start=True, stop=True)
                                dst = out_sb[0:msz, bi, mc, :]
                                if bi % 2 == 0:
                                    nc.vector.tensor_scalar(
                                        dst, ps_e[0:msz, :],
                                        bias_t[0:msz, mc:mc + 1], 0.0,
                                        mybir.AluOpType.add,
                                        mybir.AluOpType.max)
                                else:
                                    nc.scalar.activation(
                                        dst, ps_e[0:msz, :],
                                        mybir.ActivationFunctionType.Relu,
                                        bias=bias_t[0:msz, mc:mc + 1])
                        for di in range(ndense):
                            smp = smps[di]
                            ps_c = psB.tile([128, 512], f32,
                                            name=f"psc{di}_{mc}", tag="psB")
                            k = 0
                            N_MM = 24
                            for kc, (koff, ksz) in enumerate(MB[:2]):
                                for tap in range(9):
                                    dy, dx = tap // 3, tap % 3
                                    nc.tensor.matmul(
                                        ps_c[0:msz, :],
                                        wl[0:ksz,
                                           kc * 9 * C + tap * C + moff:
                                           kc * 9 * C + tap * C + moff + msz],
                                        smp[0:ksz, kc, 1 + dy:9 + dy,
                                            dx:dx + 64],
                                        start=(k == 0), stop=(k == N_MM - 1))
                                    k += 1
                            for dx in range(3):
                                # paired kc=2 taps dy=0 (parts 0:64) and
                                # dy=1 (parts 64:128, shifted duplicate)
                                nc.tensor.matmul(
                                    ps_c[0:msz, :],
                                    wl2[0:128, dx * C + moff:
                                        dx * C + moff + msz],
                                    smp[0:128, 2, 1:9, dx:dx + 64],
                                    start=False, stop=(k == N_MM - 1))
                                k += 1
                                # single kc=2 tap dy=2
                                nc.tensor.matmul(
                                    ps_c[0:msz, :],
                                    wl[0:64,
                                       2 * 9 * C + (6 + dx) * C + moff:
                                       2 * 9 * C + (6 + dx) * C + moff + msz],
                                    smp[0:64, 2, 3:11, dx:dx + 64],
                                    start=False, stop=(k == N_MM - 1))
                                k += 1
                            nc.scalar.activation(
                                out_sb[0:msz, ndeg + di, mc, :],
                                ps_c[0:msz, :],
                                mybir.ActivationFunctionType.Relu,
                                bias=bias_t[0:msz, mc:mc + 1])
                    for si in range(NB):
                        nc.sync.dma_start(
                            out=out_d[:, si * 3 * SW:(si + 1) * 3 * SW],
                            in_=out_sb[:, si, :, :])

            if reps == 1:
                body(0)
            else:
                # manual 3x unroll inside the hardware loop: consecutive
                # repetitions overlap through the double-buffered pools and
                # the For_i all-engine barrier is amortized over 3 reps
                U = 3
                n_loop = reps // U
                with tc.For_i(0, n_loop, 1) as it:
                    for u in range(U):
                        body(u)
                for u in range(reps - n_loop * U):
                    body(u)

    nc.finalize()
    _NC_CACHE[key] = nc
    return nc


# ---------------------------------------------------------------- interface


def make_in_maps(x, source_intrinsics, target_intrinsics, source_pose,
                 target_pose, conv_w, conv_b):
    return make_plan(x, source_intrinsics, target_intrinsics, source_pose,
                     target_pose, conv_w, conv_b)


def assemble(results, slots):
    """results: list of per-core {"out": [128, NB*3*SW]} -> [B, C, H, W]."""
    out = np.zeros((B, C, H, W), dtype=np.float32)
    NBl = len(slots)
    for r in range(NCORE):
        o = np.asarray(results[r]["out"]).reshape(128, NBl, 3, STRIP, W)
        for si, gb in enumerate(slots):
            for mc, (moff, msz) in enumerate(MB):
                out[gb, moff:moff + msz, 8 * r: 8 * r + STRIP, :] = \
                    o[0:msz, si, mc].astype(np.float32)
    return out


def kernel(x, source_intrinsics, target_intrinsics, source_pose,
           target_pose, conv_w, conv_b, _reps=1):
    from concourse.bass_utils import run_bass_kernel_spmd
    in_maps, struct, slots = make_in_maps(
        x, source_intrinsics, target_intrinsics, source_pose,
        target_pose, conv_w, conv_b)
    nc = build_program(_reps, struct)
    res = run_bass_kernel_spmd(nc, in_maps, list(range(NCORE)))
    return assemble(res.results, slots)


# revision 49
# speedup vs baseline: 1.4620x; 1.4620x over previous
"""Trainium2 Bass kernel for nn_EpipolarWarpOperator (B=8, C=320, H=W=64, S=3).

Sharding: spatial — every core computes an 8-row horizontal strip of all 8
batch outputs (the work of a batch is dominated by the 3x3 conv, which is
strip-separable; the epipolar sampling is handled per strip with halo).

Host analysis classifies each batch by its epipolar sampling map:

* pattern batches: the per-pixel bilinear sampling signature map has few
  distinct values (the epipolar lines clip to a handful of source locations),
  so the sampled image is piecewise constant and each output column is one of
  npat distinct "pattern" columns (3x3 signature neighborhoods incl. the
  zero-pad border). Per batch the device computes R[k, tap, pi] (replicated
  sampled columns) from a host-gathered slab via tiny matmuls,
  T^T[pi, m] = sum_{tap,kc} R_tap.T @ W_tap (+bias via a ones-row), relu on
  ACT, then expands out = T^T.T @ E with a per-core 0/1 selection matrix E.
  All x-dependent math stays on device.

* dense batches: per (128-pixel block, sample) the host pre-gathers the
  <=128 distinct bilinear corner pixels into a slab [128, 320] plus a
  sampling matrix S [128, 128]; the device runs swapped-operand matmuls
  slab.T @ S accumulating all samples (and sub-slabs) into PSUM, directly in
  channel-major order, then a 27-matmul 3x3 conv over the strip (+halo row)
  with bias+relu on ACT.
"""

import numpy as np

B, C, H, W = 8, 320, 64, 64
SN = 3
HW = H * W
STRIP = 8             # output rows per core
NCORE = 8
MB = [(0, 128), (128, 128), (256, 64)]   # output/input channel chunking
PI_MAX = 126          # total pattern budget (K of the expansion matmul)
NPAT_MAX = 56         # per-batch pattern cap (9*npat*4B must fit a PSUM bank)
NSIG_MAX = 64
NBLK = 6              # 2-row sampling blocks per strip (incl. 2-row halo pad)

# ---------------------------------------------------------------- host: geometry


def _rodrigues_np(rv):
    theta = np.sqrt((rv * rv).sum())
    r = rv / max(theta, 1e-12)
    I = np.eye(3, dtype=np.float32)
    K = np.array([[0, -r[2], r[1]], [r[2], 0, -r[0]], [-r[1], r[0], 0]],
                 dtype=np.float32)
    R = np.cos(theta) * I + (1 - np.cos(theta)) * np.outer(r, r) + np.sin(theta) * K
    return I if theta < 1e-6 else R


def fundamental_np(Ks, Kt, ps, pt):
    Fs = []
    for b in range(Ks.shape[0]):
        Rs = _rodrigues_np(ps[b, :3].astype(np.float32))
        Rt = _rodrigues_np(pt[b, :3].astype(np.float32))
        ts_, tt_ = ps[b, 3:].astype(np.float32), pt[b, 3:].astype(np.float32)
        R_rel = Rs @ Rt.T
        t_rel = ts_ - R_rel @ tt_
        z = np.float32(0)
        skew = np.array([[z, -t_rel[2], t_rel[1]],
                         [t_rel[2], z, -t_rel[0]],
                         [-t_rel[1], t_rel[0], z]], dtype=np.float32)
        E = skew @ R_rel
        inv_Ks = np.linalg.inv(Ks[b].astype(np.float32))
        inv_Kt = np.linalg.inv(Kt[b].astype(np.float32))
        Fs.append(inv_Kt.T @ E @ inv_Ks)
    return np.stack(Fs).astype(np.float32)


def geometry(F):
    k = np.arange(HW)
    px = (k % W).astype(np.float32)
    py = (k // W).astype(np.float32)
    P = np.stack([px, py, np.ones_like(px)])
    lines = F.T.astype(np.float32) @ P
    a, b_, c = lines[0], lines[1], lines[2]
    W1, H1 = np.float32(W - 1), np.float32(H - 1)
    EPS = np.float32(1e-10)
    x1 = np.clip(-c / (a + EPS), 0.0, W1)
    x2 = np.clip(-(b_ * H1 + c) / (a + EPS), 0.0, W1)
    y1 = np.clip(-c / (b_ + EPS), 0.0, H1)
    y2 = np.clip(-(a * W1 + c) / (b_ + EPS), 0.0, H1)
    t = np.linspace(0.0, 1.0, SN, dtype=np.float32)
    sx = x1[:, None] * (1 - t) + x2[:, None] * t
    sy = y1[:, None] * (1 - t) + y2[:, None] * t
    x0 = np.floor(sx)
    y0 = np.floor(sy)
    wx = (sx - x0).astype(np.float32)
    wy = (sy - y0).astype(np.float32)
    x0i = np.clip(x0, 0, W - 1).astype(np.int32)
    y0i = np.clip(y0, 0, H - 1).astype(np.int32)
    return x0i, y0i, wx, wy


def _corners(geo, p, s):
    """4 bilinear (source pixel row, weight/3) pairs for pixel p, sample s."""
    x0i, y0i, wx, wy = geo
    y0 = int(y0i[p, s]); x0 = int(x0i[p, s])
    x1 = min(x0 + 1, W - 1); y1 = min(y0 + 1, H - 1)
    wxx = np.float32(wx[p, s]); wyy = np.float32(wy[p, s])
    third = np.float32(1.0 / 3.0)
    out = {}
    for ry, rx, ww in ((y0, x0, (1 - wxx) * (1 - wyy)),
                       (y0, x1, wxx * (1 - wyy)),
                       (y1, x0, (1 - wxx) * wyy),
                       (y1, x1, wxx * wyy)):
        rr = ry * W + rx
        out[rr] = out.get(rr, np.float32(0)) + ww * third
    return out


# ------------------------------------------------------------- host: classify


def classify(geo):
    """If the sampling map has <= NSIG_MAX distinct per-pixel signatures,
    return (siginv [HW], tables); else None."""
    x0i, y0i, wx, wy = geo
    key = np.concatenate([
        y0i.astype(np.float32), x0i.astype(np.float32), wx, wy], axis=1)
    kview = np.ascontiguousarray(key).view([('', key.dtype)] * key.shape[1]).ravel()
    uniq, first, inv = np.unique(kview, return_index=True, return_inverse=True)
    if len(uniq) > NSIG_MAX:
        return None
    tables = []
    for si in range(len(uniq)):
        p = int(first[si])
        contrib = {}
        for s in range(SN):
            for rr, ww in _corners(geo, p, s).items():
                contrib[rr] = contrib.get(rr, np.float32(0)) + ww
        tables.append(sorted(contrib.items()))
    return inv.astype(np.int32), tables


def patterns(siginv):
    """3x3 neighborhood patterns of the signature map (border = -1)."""
    simg = siginv.reshape(H, W)
    pad = np.pad(simg, 1, constant_values=-1)
    neigh = np.stack([pad[dy:dy + H, dx:dx + W].ravel()
                      for dy in range(3) for dx in range(3)], axis=1)
    uniq, pinv = np.unique(neigh, axis=0, return_inverse=True)
    return pinv.astype(np.int32), uniq


# ------------------------------------------------------------ host: build plan


def _deg_slabs(entries):
    """Assign signatures (from possibly several batches) to slabs so each
    slab's source-pixel union <= 128. entries: [(bloc, si, table)] with
    table = [(pixrow, w)]. Returns list of (rowmap, sigs): rowmap =
    {(bloc, pixrow): slabrow}, sigs = [(bloc, si)]."""
    slabs = []
    rows, sigs = {}, []
    for bloc, si, tab in entries:
        new = [(bloc, rr) for rr, _ in tab if (bloc, rr) not in rows]
        if len(rows) + len(new) > 128:
            slabs.append((rows, sigs))
            rows, sigs = {}, []
            new = [(bloc, rr) for rr, _ in tab]
        for key in new:
            rows[key] = len(rows)
        sigs.append((bloc, si))
    slabs.append((rows, sigs))
    return slabs


def _dense_block_slabs(pix_ids, geo):
    """Greedy split of a 128-pixel block's (pixel, sample) tokens into slabs
    of <=128 distinct corner pixels, tokens sorted by corner location so
    nearby tokens share slab rows. Returns list of (rowmap, S[128,128])."""
    x0i, y0i = geo[0], geo[1]
    toks = [(int(y0i[p, s]), int(x0i[p, s]), i, p, s)
            for i, p in enumerate(pix_ids) if p >= 0 for s in range(SN)]
    toks.sort()
    slabs = []
    rows = {}
    S = np.zeros((128, 128), dtype=np.float32)
    for _, _, i, p, s in toks:
        cs = _corners(geo, p, s)
        new = [rr for rr in cs if rr not in rows]
        if len(rows) + len(new) > 128:
            slabs.append((rows, S))
            rows = {}
            S = np.zeros((128, 128), dtype=np.float32)
            new = list(cs)
        for rr in new:
            rows[rr] = len(rows)
        for rr, ww in cs.items():
            S[rows[rr], i] += ww
    if rows or not slabs:
        slabs.append((rows, S))
    return slabs


def make_plan(x, source_intrinsics, target_intrinsics, source_pose,
              target_pose, conv_w, conv_b):
    Fs = fundamental_np(np.asarray(source_intrinsics, np.float32),
                        np.asarray(target_intrinsics, np.float32),
                        np.asarray(source_pose, np.float32),
                        np.asarray(target_pose, np.float32))
    x = np.asarray(x, np.float32)
    xT16 = [np.ascontiguousarray(x[b].reshape(C, HW).T).astype(np.float16)
            for b in range(B)]

    degs, denses = [], []
    pi_used = 0
    for b in range(B):
        geo = geometry(Fs[b])
        res = classify(geo)
        if res is not None:
            siginv, tables = res
            pinv, ptab = patterns(siginv)
            npat = ptab.shape[0]
            if npat <= NPAT_MAX and pi_used + npat <= PI_MAX:
                degs.append(dict(gb=b, siginv=siginv, tables=tables,
                                 pinv=pinv, ptab=ptab, pi_off=pi_used))
                pi_used += npat
                continue
        denses.append(dict(gb=b, geo=geo))
    pi_tot = pi_used

    # ---- degenerate global data (same for all cores) ----
    # pack consecutive degenerate batches into groups with sum 9*npat <= 512
    # (one PSUM bank); each group's signatures are slab-packed jointly so
    # tiny batches share slabs, with column layout pi-major per batch
    import os as _os
    RGROUP_CAP = int(_os.environ.get("K_RGCAP", "512"))
    rgroups = []          # dicts: bis (deg indices), ncols, slabs
    cur, cur_n = [], 0
    for bi, d in enumerate(degs):
        n9 = 9 * d['ptab'].shape[0]
        if cur and cur_n + n9 > RGROUP_CAP:
            rgroups.append(dict(bis=cur, ncols=cur_n))
            cur, cur_n = [], 0
        cur.append(bi)
        cur_n += n9
    if cur:
        rgroups.append(dict(bis=cur, ncols=cur_n))

    sg_blocks, sr_blocks = [], []
    for g in rgroups:
        entries = []
        for bloc, bi in enumerate(g['bis']):
            for si, tab in enumerate(degs[bi]['tables']):
                entries.append((bloc, si, tab))
        slabs = _deg_slabs(entries)
        g['nslab'] = len(slabs)
        g['q0'] = degs[g['bis'][0]]['pi_off']
        g['nq'] = sum(degs[bi]['ptab'].shape[0] for bi in g['bis'])
        # group columns are tap-major (col = tap*nq + qlocal) so the PSUM
        # drain into rsb[.., kc, tap, q] is a plain contiguous-inner copy
        nq = g['nq']
        for rows, sigs in slabs:
            slab = np.zeros((128, C), dtype=np.float16)
            for (bloc, rr), idx in rows.items():
                slab[idx] = xT16[degs[g['bis'][bloc]]['gb']][rr]
            SR = np.zeros((128, g['ncols']), dtype=np.float32)
            sigset = set(sigs)
            for bloc, bi in enumerate(g['bis']):
                d = degs[bi]
                ptab = d['ptab']
                qb = d['pi_off'] - g['q0']
                for pi in range(ptab.shape[0]):
                    for tap in range(9):
                        si = ptab[pi, tap]
                        if si >= 0 and (bloc, si) in sigset:
                            for rr, ww in d['tables'][si]:
                                SR[rows[(bloc, rr)],
                                   tap * nq + qb + pi] += ww
            sg_blocks.append(slab)
            sr_blocks.append(SR.astype(np.float16))
    sg = (np.concatenate(sg_blocks, axis=1) if sg_blocks
          else np.zeros((128, 0), np.float16))
    sr = (np.concatenate(sr_blocks, axis=1) if sr_blocks
          else np.zeros((128, 0), np.float16))

    # ---- dense per-core blocks ----
    # block = 2 image rows 8r-2+2*blk, +1 of a dense batch (halo included;
    # out-of-image rows get zero columns); all 3 samples share its slabs
    core_units = []   # [core][block] -> list of (rowmap, S)
    for r in range(NCORE):
        units = []
        for d in denses:
            for blk in range(NBLK):
                row0 = 8 * r - 2 + 2 * blk
                pix = []
                for lr in range(2):
                    row = row0 + lr
                    pix += [row * W + cx if 0 <= row < H else -1
                            for cx in range(W)]
                units.append(_dense_block_slabs(pix, d['geo']))
        core_units.append(units)
    nunits = len(core_units[0])
    unit_nslab = [max(1, max(len(core_units[r][u]) for r in range(NCORE)))
                  for u in range(nunits)]

    ui = 0
    for d in denses:
        d['unit_nslab'] = tuple(unit_nslab[ui:ui + NBLK])
        ui += NBLK

    # ---- weights ----
    Wl = np.zeros((128, 3 * 9 * C), dtype=np.float16)
    cw = np.asarray(conv_w, np.float32)
    cb = np.asarray(conv_b, np.float32)
    for kc, (koff, ksz) in enumerate(MB):
        for tap in range(9):
            dy, dx = tap // 3, tap % 3
            Wl[0:ksz, kc * 9 * C + tap * C: kc * 9 * C + tap * C + C] = \
                cw[:, koff:koff + ksz, dy, dx].T.astype(np.float16)
    # paired kc=2 weights for the dense conv: partitions 0:64 hold the
    # dy=0 tap, 64:128 the dy=1 tap (read through the +1-row shifted
    # duplicate of the kc=2 sampled plane)
    Wl2 = np.zeros((128, 3 * C), dtype=np.float16)
    for dxi in range(3):
        Wl2[0:64, dxi * C: dxi * C + C] = \
            cw[:, 256:320, 0, dxi].T.astype(np.float16)
        Wl2[64:128, dxi * C: dxi * C + C] = \
            cw[:, 256:320, 1, dxi].T.astype(np.float16)
    bias = np.zeros((128, 3), dtype=np.float32)
    for mc, (moff, msz) in enumerate(MB):
        bias[0:msz, mc] = cb[moff:moff + msz]

    # ---- per-core in_maps ----
    ndeg, ndense = len(degs), len(denses)
    slots = [d['gb'] for d in degs] + [d['gb'] for d in denses]
    in_maps = []
    for r in range(NCORE):
        m = {"wl": Wl, "wl2": Wl2, "bias": bias}
        if ndeg:
            m["sg"] = sg
            m["sr"] = sr
            e = np.zeros((128, ndeg * STRIP * W), dtype=np.float16)
            for bi, d in enumerate(degs):
                pidx = d['pinv'].reshape(H, W)[8 * r: 8 * r + STRIP].ravel()
                e[d['pi_off'] + pidx,
                  bi * STRIP * W + np.arange(STRIP * W)] = 1.0
            m["e_mat"] = e
        if ndense:
            sds, sss = [], []
            for u in range(nunits):
                slabs = core_units[r][u]
                di = u // NBLK
                gb = denses[di]['gb']
                for j in range(unit_nslab[u]):
                    slab = np.zeros((128, C), dtype=np.float16)
                    S = np.zeros((128, 128), dtype=np.float16)
                    if j < len(slabs):
                        rows, Sf = slabs[j]
                        rl = sorted(rows, key=rows.get)
                        if rl:
                            slab[:len(rl)] = xT16[gb][np.array(rl)]
                        S = Sf.astype(np.float16)
                    sds.append(slab)
                    sss.append(S)
            m["sd"] = np.concatenate(sds, axis=1)
            m["ss"] = np.concatenate(sss, axis=1)
        in_maps.append(m)

    struct = (pi_tot,
              tuple((d['gb'], d['ptab'].shape[0]) for d in degs),
              tuple((g['ncols'], g['nslab'], g['q0'], g['nq'])
                    for g in rgroups),
              tuple((d['gb'], d['unit_nslab']) for d in denses))
    return in_maps, struct, slots


# ------------------------------------------------------------- bass program

_NC_CACHE = {}


def build_program(reps, struct):
    key = (reps, struct)
    if key in _NC_CACHE:
        return _NC_CACHE[key]
    import concourse.bacc as bacc
    import concourse.mybir as mybir
    from concourse.tile import TileContext

    fp16 = mybir.dt.float16
    f32 = mybir.dt.float32
    pi_tot, degs, rgroups, denses = struct
    ndeg, ndense = len(degs), len(denses)
    NB = ndeg + ndense
    NSG = sum(ns for _, ns, _, _ in rgroups)
    SRC = sum(nc_ * ns for nc_, ns, _, _ in rgroups)
    NSLAB = sum(sum(us) for _, us in denses)
    SW = STRIP * W   # 512 pixels per strip

    nc = bacc.Bacc(target_bir_lowering=False)
    wl_d = nc.dram_tensor("wl", [128, 3 * 9 * C], fp16, kind="ExternalInput")
    wl2_d = nc.dram_tensor("wl2", [128, 3 * C], fp16, kind="ExternalInput")
    bias_d = nc.dram_tensor("bias", [128, 3], f32, kind="ExternalInput")
    if ndeg:
        sg_d = nc.dram_tensor("sg", [128, NSG * C], fp16, kind="ExternalInput")
        sr_d = nc.dram_tensor("sr", [128, SRC], fp16, kind="ExternalInput")
        e_d = nc.dram_tensor("e_mat", [128, ndeg * SW], fp16,
                             kind="ExternalInput")
    if ndense:
        sd_d = nc.dram_tensor("sd", [128, NSLAB * C], fp16,
                              kind="ExternalInput")
        ss_d = nc.dram_tensor("ss", [128, NSLAB * 128], fp16,
                              kind="ExternalInput")
    out_d = nc.dram_tensor("out", [128, NB * 3 * SW], fp16,
                           kind="ExternalOutput")

    with TileContext(nc) as tc:
        with tc.tile_pool(name="const", bufs=1) as constp, \
             tc.tile_pool(name="inp", bufs=2) as inp, \
             tc.tile_pool(name="sdp", bufs=2) as sdp, \
             tc.tile_pool(name="ssp", bufs=2) as ssp, \
             tc.tile_pool(name="work", bufs=2) as work, \
             tc.tile_pool(name="smpp", bufs=2) as smpp, \
             tc.tile_pool(name="outp", bufs=2) as outp, \
             tc.tile_pool(name="psA", bufs=3, space="PSUM") as psA, \
             tc.tile_pool(name="psB", bufs=5, space="PSUM") as psB:
            wl = constp.tile([128, 3 * 9 * C], fp16)
            nc.sync.dma_start(out=wl[:], in_=wl_d[:])
            wl2 = constp.tile([128, 3 * C], fp16)
            nc.sync.dma_start(out=wl2[:], in_=wl2_d[:])
            bias_t = constp.tile([128, 3], f32)
            nc.sync.dma_start(out=bias_t[:], in_=bias_d[:])

            def body(_it):
                if True:
                    out_sb = outp.tile([128, NB, 3, SW], fp16, name="out_sb",
                                       tag="out_sb")
                    # mc=2 has only 64 valid channel partitions; zero the rest
                    # so the out DMA never reads uninitialized SBUF
                    nc.gpsimd.memset(out_sb[64:128, :, 2:3, :], 0.0)

                    # ---------- input DMAs ----------
                    if ndeg:
                        sg = inp.tile([128, NSG * C], fp16, name="sg",
                                      tag="sg")
                        nc.sync.dma_start(out=sg[:], in_=sg_d[:])
                        sr = inp.tile([128, SRC], fp16, name="sr", tag="sr")
                        nc.sync.dma_start(out=sr[:], in_=sr_d[:])
                    sd_tiles, ss_tiles = [], []
                    if ndense:
                        # chunk slab streams per (dense batch, block pair)
                        off = 0
                        for di, (_, us) in enumerate(denses):
                            for b2 in range(0, NBLK, 2):
                                nsl = us[b2] + us[b2 + 1]
                                sdt = sdp.tile([128, nsl * C], fp16,
                                               tag=f"sd{b2}")
                                nc.sync.dma_start(
                                    out=sdt[:],
                                    in_=sd_d[:, off * C:(off + nsl) * C])
                                sst = ssp.tile([128, nsl * 128], fp16,
                                               tag=f"ss{b2}")
                                nc.sync.dma_start(
                                    out=sst[:],
                                    in_=ss_d[:, off * 128:(off + nsl) * 128])
                                sd_tiles += [(sdt, 0), (sdt, us[b2])]
                                ss_tiles += [(sst, 0), (sst, us[b2])]
                                off += nsl
                    if ndeg:
                        # needed only by the late expansion matmuls, and only
                        # rows 0:pi_tot — issued after the sampling streams
                        e = inp.tile([128, ndeg * SW], fp16, name="e",
                                     tag="e")
                        nc.sync.dma_start(out=e[0:pi_tot, :],
                                          in_=e_d[0:pi_tot, :])

                    # ---------- degenerate path: R ----------
                    if ndeg:
                        rsb = work.tile([128, 3, 9, pi_tot], fp16, name="rsb",
                                        tag="rsb")
                        sgo, sro = 0, 0
                        for gi, (ncols, nsl, q0, nq) in enumerate(rgroups):
                            for kc, (koff, ksz) in enumerate(MB):
                                ps_r = psA.tile([128, 512], f32,
                                                name=f"psr{gi}_{kc}",
                                                tag="psA")
                                for j in range(nsl):
                                    nc.tensor.matmul(
                                        ps_r[0:ksz, 0:ncols],
                                        sg[:, (sgo + j) * C + koff:
                                           (sgo + j) * C + koff + ksz],
                                        sr[:, sro + j * ncols:
                                           sro + (j + 1) * ncols],
                                        start=(j == 0), stop=(j == nsl - 1))
                                psq = ps_r[:, 0:ncols].rearrange(
                                    "p (t q) -> p t q", t=9)
                                nc.vector.tensor_copy(
                                    rsb[0:ksz, kc, :, q0:q0 + nq],
                                    psq[0:ksz])
                            sgo += nsl
                            sro += nsl * ncols

                    # ---------- dense sampling (first half) ----------
                    smps = []
                    if ndense:
                        for di in range(ndense):
                            smp = smpp.tile([128, 3, 2 * NBLK, 66], fp16,
                                            name=f"smp{di}", tag=f"smp{di}")
                            nc.gpsimd.memset(smp[:, :, :, 0:1], 0.0)
                            nc.gpsimd.memset(smp[:, :, :, 65:66], 0.0)
                            smps.append(smp)

                    def dense_block(di, blk):
                        _, us = denses[di]
                        smp = smps[di]
                        sdt, sdo = sd_tiles[di * NBLK + blk]
                        sst, sso = ss_tiles[di * NBLK + blk]
                        ps = psA.tile([128, 512], f32,
                                      name=f"psb{di}_{blk}", tag="psA")
                        nsl = us[blk]
                        for kc, (koff, ksz) in enumerate(MB):
                            for k in range(nsl):
                                nc.tensor.matmul(
                                    ps[0:ksz, kc * 128:(kc + 1) * 128],
                                    sdt[:, (sdo + k) * C + koff:
                                        (sdo + k) * C + koff + ksz],
                                    sst[:, (sso + k) * 128:
                                        (sso + k + 1) * 128],
                                    start=(k == 0), stop=(k == nsl - 1))
                        psv = ps[:, 0:384].rearrange("p (k r c) -> p k r c",
                                                     k=3, r=2)
                        nc.vector.tensor_copy(
                            smp[0:128, 0:2, 2 * blk:2 * blk + 2, 1:65],
                            psv[0:128, 0:2, :, :])
                        nc.vector.tensor_copy(
                            smp[0:64, 2:3, 2 * blk:2 * blk + 2, 1:65],
                            psv[0:64, 2:3, :, :])
                        if blk >= 1:
                            # +1-row shifted duplicate of the kc=2 plane in
                            # partitions 64:128, for the paired conv taps
                            nc.vector.tensor_copy(
                                smp[64:128, 2:3, 2 * blk - 1:2 * blk + 1,
                                    1:65],
                                psv[0:64, 2:3, :, :])

                    if ndense:
                        for di in range(ndense):
                            for blk in range(3):
                                dense_block(di, blk)

                    # ---------- degenerate path: T ----------
                    # bias + relu commute with the per-pixel column selection,
                    # so they are applied after the expansion matmul instead
                    if ndeg:
                        ps_t = psB.tile([128, 512], f32, name="ps_t",
                                        tag="psB")
                        k = 0
                        for kc, (koff, ksz) in enumerate(MB):
                            for tap in range(9):
                                nc.tensor.matmul(
                                    ps_t[0:pi_tot, 0:C],
                                    rsb[0:ksz, kc, tap, :],
                                    wl[0:ksz, kc * 9 * C + tap * C:
                                       kc * 9 * C + tap * C + C],
                                    start=(k == 0), stop=(k == 26))
                                k += 1
                        tsb = work.tile([128, C], fp16, name="tsb", tag="tsb")
                        nc.scalar.copy(tsb[0:pi_tot, :], ps_t[0:pi_tot, 0:C])

                    # ---------- dense sampling (second half) ----------
                    if ndense:
                        for di in range(ndense):
                            for blk in range(3, NBLK):
                                dense_block(di, blk)

                    # ---------- expansion + dense conv, interleaved per mc ----
                    # the conv matmul groups run on PE while the expansion
                    # PSUM drains complete on DVE/ACT
                    for mc, (moff, msz) in enumerate(MB):
                        if ndeg:
                            for bi in range(ndeg):
                                ps_e = psB.tile([128, 512], f32,
                                                name=f"pse{mc}_{bi}",
                                                tag="psB")
                                nc.tensor.matmul(
                                    ps_e[0:msz, :],
                                    tsb[0:pi_tot, moff:moff + msz],
                                    e[0:pi_tot, bi * SW:(bi + 1) * SW],
                                    start=True, stop=True)
                                dst = out_sb[0:msz, bi, mc, :]
                                if bi % 2 == 0:
                                    nc.vector.tensor_scalar(
                                        dst, ps_e[0:msz, :],
                                        bias_t[0:msz, mc:mc + 1], 0.0,
                                        mybir.AluOpType.add,
                                        mybir.AluOpType.max)
                                else:
                                    nc.scalar.activation(
                                        dst, ps_e[0:msz, :],
                                        mybir.ActivationFunctionType.Relu,
                                        bias=bias_t[0:msz, mc:mc + 1])
                        for di in range(ndense):
                            smp = smps[di]
                            ps_c = psB.tile([128, 512], f32,
                                            name=f"psc{di}_{mc}", tag="psB")
                            k = 0
                            N_MM = 24
                            for kc, (koff, ksz) in enumerate(MB[:2]):
                                for tap in range(9):
                                    dy, dx = tap // 3, tap % 3
                                    nc.tensor.matmul(
                                        ps_c[0:msz, :],
                                        wl[0:ksz,
                                           kc * 9 * C + tap * C + moff:
                                           kc * 9 * C + tap * C + moff + msz],
                                        smp[0:ksz, kc, 1 + dy:9 + dy,
                                            dx:dx + 64],
                                        start=(k == 0), stop=(k == N_MM - 1))
                                    k += 1
                            for dx in range(3):
                                # paired kc=2 taps dy=0 (parts 0:64) and
                                # dy=1 (parts 64:128, shifted duplicate)
                                nc.tensor.matmul(
                                    ps_c[0:msz, :],
                                    wl2[0:128, dx * C + moff:
                                        dx * C + moff + msz],
                                    smp[0:128, 2, 1:9, dx:dx + 64],
                                    start=False, stop=(k == N_MM - 1))
                                k += 1
                                # single kc=2 tap dy=2
                                nc.tensor.matmul(
                                    ps_c[0:msz, :],
                                    wl[0:64,
                                       2 * 9 * C + (6 + dx) * C + moff:
                                       2 * 9 * C + (6 + dx) * C + moff + msz],
                                    smp[0:64, 2, 3:11, dx:dx + 64],
                                    start=False, stop=(k == N_MM - 1))
                                k += 1
                            nc.scalar.activation(
                                out_sb[0:msz, ndeg + di, mc, :],
                                ps_c[0:msz, :],
                                mybir.ActivationFunctionType.Relu,
                                bias=bias_t[0:msz, mc:mc + 1])
                    for si in range(NB):
                        nc.sync.dma_start(
                            out=out_d[:, si * 3 * SW:(si + 1) * 3 * SW],
                            in_=out_sb[:, si, :, :])

            if reps == 1:
                body(0)
            else:
                # manual 3x unroll inside the hardware loop: consecutive
                # repetitions overlap through the double-buffered pools and
                # the For_i all-engine barrier is amortized over 3 reps
                U = 3
                n_loop = reps // U
                hints = (mybir.EngineType.PE, mybir.EngineType.Activation,
                         mybir.EngineType.Pool, mybir.EngineType.SP,
                         mybir.EngineType.DVE)
                with tc.For_i(0, n_loop, 1, hint_engines=hints) as it:
                    for u in range(U):
                        body(u)
                for u in range(reps - n_loop * U):
                    body(u)

    nc.finalize()
    _NC_CACHE[key] = nc
    return nc


# ---------------------------------------------------------------- interface


def make_in_maps(x, source_intrinsics, target_intrinsics, source_pose,
                 target_pose, conv_w, conv_b):
    return make_plan(x, source_intrinsics, target_intrinsics, source_pose,
                     target_pose, conv_w, conv_b)


def assemble(results, slots):
    """results: list of per-core {"out": [128, NB*3*SW]} -> [B, C, H, W]."""
    out = np.zeros((B, C, H, W), dtype=np.float32)
    NBl = len(slots)
    for r in range(NCORE):
        o = np.asarray(results[r]["out"]).reshape(128, NBl, 3, STRIP, W)
        for si, gb in enumerate(slots):
            for mc, (moff, msz) in enumerate(MB):
                out[gb, moff:moff + msz, 8 * r: 8 * r + STRIP, :] = \
                    o[0:msz, si, mc].astype(np.float32)
    return out


def kernel(x, source_intrinsics, target_intrinsics, source_pose,
           target_pose, conv_w, conv_b, _reps=1):
    from concourse.bass_utils import run_bass_kernel_spmd
    in_maps, struct, slots = make_in_maps(
        x, source_intrinsics, target_intrinsics, source_pose,
        target_pose, conv_w, conv_b)
    nc = build_program(_reps, struct)
    res = run_bass_kernel_spmd(nc, in_maps, list(range(NCORE)))
    return assemble(res.results, slots)


# revision 50
# speedup vs baseline: 1.6680x; 1.1409x over previous
"""Trainium2 Bass kernel for nn_EpipolarWarpOperator (B=8, C=320, H=W=64, S=3).

Sharding: spatial — every core computes an 8-row horizontal strip of all 8
batch outputs (the work of a batch is dominated by the 3x3 conv, which is
strip-separable; the epipolar sampling is handled per strip with halo).

Host analysis classifies each batch by its epipolar sampling map:

* pattern batches: the per-pixel bilinear sampling signature map has few
  distinct values (the epipolar lines clip to a handful of source locations),
  so the sampled image is piecewise constant and each output column is one of
  npat distinct "pattern" columns (3x3 signature neighborhoods incl. the
  zero-pad border). Per batch the device computes R[k, tap, pi] (replicated
  sampled columns) from a host-gathered slab via tiny matmuls,
  T^T[pi, m] = sum_{tap,kc} R_tap.T @ W_tap (+bias via a ones-row), relu on
  ACT, then expands out = T^T.T @ E with a per-core 0/1 selection matrix E.
  All x-dependent math stays on device.

* dense batches: per (128-pixel block, sample) the host pre-gathers the
  <=128 distinct bilinear corner pixels into a slab [128, 320] plus a
  sampling matrix S [128, 128]; the device runs swapped-operand matmuls
  slab.T @ S accumulating all samples (and sub-slabs) into PSUM, directly in
  channel-major order, then a 27-matmul 3x3 conv over the strip (+halo row)
  with bias+relu on ACT.
"""

import numpy as np

B, C, H, W = 8, 320, 64, 64
SN = 3
HW = H * W
STRIP = 8             # output rows per core
NCORE = 8
MB = [(0, 128), (128, 128), (256, 64)]   # output/input channel chunking
PI_MAX = 126          # total pattern budget (K of the expansion matmul)
NPAT_MAX = 56         # per-batch pattern cap (9*npat*4B must fit a PSUM bank)
NSIG_MAX = 64
NBLK = 6              # 2-row sampling blocks per strip (incl. 2-row halo pad)

# ---------------------------------------------------------------- host: geometry


def _rodrigues_np(rv):
    theta = np.sqrt((rv * rv).sum())
    r = rv / max(theta, 1e-12)
    I = np.eye(3, dtype=np.float32)
    K = np.array([[0, -r[2], r[1]], [r[2], 0, -r[0]], [-r[1], r[0], 0]],
                 dtype=np.float32)
    R = np.cos(theta) * I + (1 - np.cos(theta)) * np.outer(r, r) + np.sin(theta) * K
    return I if theta < 1e-6 else R


def fundamental_np(Ks, Kt, ps, pt):
    Fs = []
    for b in range(Ks.shape[0]):
        Rs = _rodrigues_np(ps[b, :3].astype(np.float32))
        Rt = _rodrigues_np(pt[b, :3].astype(np.float32))
        ts_, tt_ = ps[b, 3:].astype(np.float32), pt[b, 3:].astype(np.float32)
        R_rel = Rs @ Rt.T
        t_rel = ts_ - R_rel @ tt_
        z = np.float32(0)
        skew = np.array([[z, -t_rel[2], t_rel[1]],
                         [t_rel[2], z, -t_rel[0]],
                         [-t_rel[1], t_rel[0], z]], dtype=np.float32)
        E = skew @ R_rel
        inv_Ks = np.linalg.inv(Ks[b].astype(np.float32))
        inv_Kt = np.linalg.inv(Kt[b].astype(np.float32))
        Fs.append(inv_Kt.T @ E @ inv_Ks)
    return np.stack(Fs).astype(np.float32)


def geometry(F):
    k = np.arange(HW)
    px = (k % W).astype(np.float32)
    py = (k // W).astype(np.float32)
    P = np.stack([px, py, np.ones_like(px)])
    lines = F.T.astype(np.float32) @ P
    a, b_, c = lines[0], lines[1], lines[2]
    W1, H1 = np.float32(W - 1), np.float32(H - 1)
    EPS = np.float32(1e-10)
    x1 = np.clip(-c / (a + EPS), 0.0, W1)
    x2 = np.clip(-(b_ * H1 + c) / (a + EPS), 0.0, W1)
    y1 = np.clip(-c / (b_ + EPS), 0.0, H1)
    y2 = np.clip(-(a * W1 + c) / (b_ + EPS), 0.0, H1)
    t = np.linspace(0.0, 1.0, SN, dtype=np.float32)
    sx = x1[:, None] * (1 - t) + x2[:, None] * t
    sy = y1[:, None] * (1 - t) + y2[:, None] * t
    x0 = np.floor(sx)
    y0 = np.floor(sy)
    wx = (sx - x0).astype(np.float32)
    wy = (sy - y0).astype(np.float32)
    x0i = np.clip(x0, 0, W - 1).astype(np.int32)
    y0i = np.clip(y0, 0, H - 1).astype(np.int32)
    return x0i, y0i, wx, wy


def _corners(geo, p, s):
    """4 bilinear (source pixel row, weight/3) pairs for pixel p, sample s."""
    x0i, y0i, wx, wy = geo
    y0 = int(y0i[p, s]); x0 = int(x0i[p, s])
    x1 = min(x0 + 1, W - 1); y1 = min(y0 + 1, H - 1)
    wxx = np.float32(wx[p, s]); wyy = np.float32(wy[p, s])
    third = np.float32(1.0 / 3.0)
    out = {}
    for ry, rx, ww in ((y0, x0, (1 - wxx) * (1 - wyy)),
                       (y0, x1, wxx * (1 - wyy)),
                       (y1, x0, (1 - wxx) * wyy),
                       (y1, x1, wxx * wyy)):
        rr = ry * W + rx
        out[rr] = out.get(rr, np.float32(0)) + ww * third
    return out


# ------------------------------------------------------------- host: classify


def classify(geo):
    """If the sampling map has <= NSIG_MAX distinct per-pixel signatures,
    return (siginv [HW], tables); else None."""
    x0i, y0i, wx, wy = geo
    key = np.concatenate([
        y0i.astype(np.float32), x0i.astype(np.float32), wx, wy], axis=1)
    kview = np.ascontiguousarray(key).view([('', key.dtype)] * key.shape[1]).ravel()
    uniq, first, inv = np.unique(kview, return_index=True, return_inverse=True)
    if len(uniq) > NSIG_MAX:
        return None
    tables = []
    for si in range(len(uniq)):
        p = int(first[si])
        contrib = {}
        for s in range(SN):
            for rr, ww in _corners(geo, p, s).items():
                contrib[rr] = contrib.get(rr, np.float32(0)) + ww
        tables.append(sorted(contrib.items()))
    return inv.astype(np.int32), tables


def patterns(siginv):
    """3x3 neighborhood patterns of the signature map (border = -1)."""
    simg = siginv.reshape(H, W)
    pad = np.pad(simg, 1, constant_values=-1)
    neigh = np.stack([pad[dy:dy + H, dx:dx + W].ravel()
                      for dy in range(3) for dx in range(3)], axis=1)
    uniq, pinv = np.unique(neigh, axis=0, return_inverse=True)
    return pinv.astype(np.int32), uniq


# ------------------------------------------------------------ host: build plan


def _deg_slabs(entries):
    """Assign signatures (from possibly several batches) to slabs so each
    slab's source-pixel union <= 128. entries: [(bloc, si, table)] with
    table = [(pixrow, w)]. Returns list of (rowmap, sigs): rowmap =
    {(bloc, pixrow): slabrow}, sigs = [(bloc, si)]."""
    slabs = []
    rows, sigs = {}, []
    for bloc, si, tab in entries:
        new = [(bloc, rr) for rr, _ in tab if (bloc, rr) not in rows]
        if len(rows) + len(new) > 128:
            slabs.append((rows, sigs))
            rows, sigs = {}, []
            new = [(bloc, rr) for rr, _ in tab]
        for key in new:
            rows[key] = len(rows)
        sigs.append((bloc, si))
    slabs.append((rows, sigs))
    return slabs


def _dense_block_slabs(pix_ids, geo):
    """Greedy split of a 128-pixel block's (pixel, sample) tokens into slabs
    of <=128 distinct corner pixels, tokens sorted by corner location so
    nearby tokens share slab rows. Returns list of (rowmap, S[128,128])."""
    x0i, y0i = geo[0], geo[1]
    toks = [(int(y0i[p, s]), int(x0i[p, s]), i, p, s)
            for i, p in enumerate(pix_ids) if p >= 0 for s in range(SN)]
    toks.sort()
    slabs = []
    rows = {}
    S = np.zeros((128, 128), dtype=np.float32)
    for _, _, i, p, s in toks:
        cs = _corners(geo, p, s)
        new = [rr for rr in cs if rr not in rows]
        if len(rows) + len(new) > 128:
            slabs.append((rows, S))
            rows = {}
            S = np.zeros((128, 128), dtype=np.float32)
            new = list(cs)
        for rr in new:
            rows[rr] = len(rows)
        for rr, ww in cs.items():
            S[rows[rr], i] += ww
    if rows or not slabs:
        slabs.append((rows, S))
    return slabs


def make_plan(x, source_intrinsics, target_intrinsics, source_pose,
              target_pose, conv_w, conv_b):
    Fs = fundamental_np(np.asarray(source_intrinsics, np.float32),
                        np.asarray(target_intrinsics, np.float32),
                        np.asarray(source_pose, np.float32),
                        np.asarray(target_pose, np.float32))
    x = np.asarray(x, np.float32)
    xT16 = [np.ascontiguousarray(x[b].reshape(C, HW).T).astype(np.float16)
            for b in range(B)]

    degs, denses = [], []
    pi_used = 0
    for b in range(B):
        geo = geometry(Fs[b])
        res = classify(geo)
        if res is not None:
            siginv, tables = res
            pinv, ptab = patterns(siginv)
            npat = ptab.shape[0]
            if npat <= NPAT_MAX and pi_used + npat <= PI_MAX:
                degs.append(dict(gb=b, siginv=siginv, tables=tables,
                                 pinv=pinv, ptab=ptab, pi_off=pi_used))
                pi_used += npat
                continue
        denses.append(dict(gb=b, geo=geo))
    pi_tot = pi_used

    # ---- degenerate global data (same for all cores) ----
    # pack consecutive degenerate batches into groups with sum 9*npat <= 512
    # (one PSUM bank); each group's signatures are slab-packed jointly so
    # tiny batches share slabs, with column layout pi-major per batch
    import os as _os
    RGROUP_CAP = int(_os.environ.get("K_RGCAP", "512"))
    rgroups = []          # dicts: bis (deg indices), ncols, slabs
    cur, cur_n = [], 0
    for bi, d in enumerate(degs):
        n9 = 9 * d['ptab'].shape[0]
        if cur and cur_n + n9 > RGROUP_CAP:
            rgroups.append(dict(bis=cur, ncols=cur_n))
            cur, cur_n = [], 0
        cur.append(bi)
        cur_n += n9
    if cur:
        rgroups.append(dict(bis=cur, ncols=cur_n))

    sg_blocks, sr_blocks = [], []
    for g in rgroups:
        entries = []
        for bloc, bi in enumerate(g['bis']):
            for si, tab in enumerate(degs[bi]['tables']):
                entries.append((bloc, si, tab))
        slabs = _deg_slabs(entries)
        g['nslab'] = len(slabs)
        g['q0'] = degs[g['bis'][0]]['pi_off']
        g['nq'] = sum(degs[bi]['ptab'].shape[0] for bi in g['bis'])
        # group columns are tap-major (col = tap*nq + qlocal) so the PSUM
        # drain into rsb[.., kc, tap, q] is a plain contiguous-inner copy
        nq = g['nq']
        for rows, sigs in slabs:
            slab = np.zeros((128, C), dtype=np.float16)
            for (bloc, rr), idx in rows.items():
                slab[idx] = xT16[degs[g['bis'][bloc]]['gb']][rr]
            SR = np.zeros((128, g['ncols']), dtype=np.float32)
            sigset = set(sigs)
            for bloc, bi in enumerate(g['bis']):
                d = degs[bi]
                ptab = d['ptab']
                qb = d['pi_off'] - g['q0']
                for pi in range(ptab.shape[0]):
                    for tap in range(9):
                        si = ptab[pi, tap]
                        if si >= 0 and (bloc, si) in sigset:
                            for rr, ww in d['tables'][si]:
                                SR[rows[(bloc, rr)],
                                   tap * nq + qb + pi] += ww
            sg_blocks.append(slab)
            sr_blocks.append(SR.astype(np.float16))
    sg = (np.concatenate(sg_blocks, axis=1) if sg_blocks
          else np.zeros((128, 0), np.float16))
    sr = (np.concatenate(sr_blocks, axis=1) if sr_blocks
          else np.zeros((128, 0), np.float16))

    # ---- dense per-core blocks ----
    # block = 2 image rows 8r-2+2*blk, +1 of a dense batch (halo included;
    # out-of-image rows get zero columns); all 3 samples share its slabs
    core_units = []   # [core][block] -> list of (rowmap, S)
    for r in range(NCORE):
        units = []
        for d in denses:
            for blk in range(NBLK):
                row0 = 8 * r - 2 + 2 * blk
                pix = []
                for lr in range(2):
                    row = row0 + lr
                    pix += [row * W + cx if 0 <= row < H else -1
                            for cx in range(W)]
                units.append(_dense_block_slabs(pix, d['geo']))
        core_units.append(units)
    nunits = len(core_units[0])
    unit_nslab = [max(1, max(len(core_units[r][u]) for r in range(NCORE)))
                  for u in range(nunits)]

    ui = 0
    for d in denses:
        d['unit_nslab'] = tuple(unit_nslab[ui:ui + NBLK])
        ui += NBLK

    # ---- weights ----
    Wl = np.zeros((128, 3 * 9 * C), dtype=np.float16)
    cw = np.asarray(conv_w, np.float32)
    cb = np.asarray(conv_b, np.float32)
    for kc, (koff, ksz) in enumerate(MB):
        for tap in range(9):
            dy, dx = tap // 3, tap % 3
            Wl[0:ksz, kc * 9 * C + tap * C: kc * 9 * C + tap * C + C] = \
                cw[:, koff:koff + ksz, dy, dx].T.astype(np.float16)
    # paired kc=2 weights for the dense conv: partitions 0:64 hold the
    # dy=0 tap, 64:128 the dy=1 tap (read through the +1-row shifted
    # duplicate of the kc=2 sampled plane)
    Wl2 = np.zeros((128, 3 * C), dtype=np.float16)
    for dxi in range(3):
        Wl2[0:64, dxi * C: dxi * C + C] = \
            cw[:, 256:320, 0, dxi].T.astype(np.float16)
        Wl2[64:128, dxi * C: dxi * C + C] = \
            cw[:, 256:320, 1, dxi].T.astype(np.float16)
    bias = np.zeros((128, 3), dtype=np.float32)
    for mc, (moff, msz) in enumerate(MB):
        bias[0:msz, mc] = cb[moff:moff + msz]

    # ---- per-core in_maps ----
    ndeg, ndense = len(degs), len(denses)
    slots = [d['gb'] for d in degs] + [d['gb'] for d in denses]
    in_maps = []
    for r in range(NCORE):
        m = {"wl": Wl, "wl2": Wl2, "bias": bias}
        if ndeg:
            m["sg"] = sg
            m["sr"] = sr
            e = np.zeros((128, ndeg * STRIP * W), dtype=np.float16)
            for bi, d in enumerate(degs):
                pidx = d['pinv'].reshape(H, W)[8 * r: 8 * r + STRIP].ravel()
                e[d['pi_off'] + pidx,
                  bi * STRIP * W + np.arange(STRIP * W)] = 1.0
            m["e_mat"] = e
        if ndense:
            sds, sss = [], []
            for u in range(nunits):
                slabs = core_units[r][u]
                di = u // NBLK
                gb = denses[di]['gb']
                for j in range(unit_nslab[u]):
                    slab = np.zeros((128, C), dtype=np.float16)
                    S = np.zeros((128, 128), dtype=np.float16)
                    if j < len(slabs):
                        rows, Sf = slabs[j]
                        rl = sorted(rows, key=rows.get)
                        if rl:
                            slab[:len(rl)] = xT16[gb][np.array(rl)]
                        S = Sf.astype(np.float16)
                    sds.append(slab)
                    sss.append(S)
            m["sd"] = np.concatenate(sds, axis=1)
            m["ss"] = np.concatenate(sss, axis=1)
        in_maps.append(m)

    struct = (pi_tot,
              tuple((d['gb'], d['ptab'].shape[0]) for d in degs),
              tuple((g['ncols'], g['nslab'], g['q0'], g['nq'])
                    for g in rgroups),
              tuple((d['gb'], d['unit_nslab']) for d in denses))
    return in_maps, struct, slots


# ------------------------------------------------------------- bass program

_NC_CACHE = {}


def build_program(reps, struct):
    key = (reps, struct)
    if key in _NC_CACHE:
        return _NC_CACHE[key]
    import concourse.bacc as bacc
    import concourse.mybir as mybir
    from concourse.tile import TileContext

    fp16 = mybir.dt.float16
    f32 = mybir.dt.float32
    pi_tot, degs, rgroups, denses = struct
    ndeg, ndense = len(degs), len(denses)
    NB = ndeg + ndense
    NSG = sum(ns for _, ns, _, _ in rgroups)
    SRC = sum(nc_ * ns for nc_, ns, _, _ in rgroups)
    NSLAB = sum(sum(us) for _, us in denses)
    SW = STRIP * W   # 512 pixels per strip

    nc = bacc.Bacc(target_bir_lowering=False)
    wl_d = nc.dram_tensor("wl", [128, 3 * 9 * C], fp16, kind="ExternalInput")
    wl2_d = nc.dram_tensor("wl2", [128, 3 * C], fp16, kind="ExternalInput")
    bias_d = nc.dram_tensor("bias", [128, 3], f32, kind="ExternalInput")
    if ndeg:
        sg_d = nc.dram_tensor("sg", [128, NSG * C], fp16, kind="ExternalInput")
        sr_d = nc.dram_tensor("sr", [128, SRC], fp16, kind="ExternalInput")
        e_d = nc.dram_tensor("e_mat", [128, ndeg * SW], fp16,
                             kind="ExternalInput")
    if ndense:
        sd_d = nc.dram_tensor("sd", [128, NSLAB * C], fp16,
                              kind="ExternalInput")
        ss_d = nc.dram_tensor("ss", [128, NSLAB * 128], fp16,
                              kind="ExternalInput")
    out_d = nc.dram_tensor("out", [128, NB * 3 * SW], fp16,
                           kind="ExternalOutput")

    with TileContext(nc) as tc:
        with tc.tile_pool(name="const", bufs=1) as constp, \
             tc.tile_pool(name="inp", bufs=2) as inp, \
             tc.tile_pool(name="sdp", bufs=2) as sdp, \
             tc.tile_pool(name="ssp", bufs=2) as ssp, \
             tc.tile_pool(name="work", bufs=2) as work, \
             tc.tile_pool(name="smpp", bufs=2) as smpp, \
             tc.tile_pool(name="outp", bufs=2) as outp, \
             tc.tile_pool(name="psA", bufs=3, space="PSUM") as psA, \
             tc.tile_pool(name="psB", bufs=5, space="PSUM") as psB:
            wl = constp.tile([128, 3 * 9 * C], fp16)
            nc.sync.dma_start(out=wl[:], in_=wl_d[:])
            wl2 = constp.tile([128, 3 * C], fp16)
            nc.sync.dma_start(out=wl2[:], in_=wl2_d[:])
            bias_t = constp.tile([128, 3], f32)
            nc.sync.dma_start(out=bias_t[:], in_=bias_d[:])

            def body(_it):
                if True:
                    out_sb = outp.tile([128, NB, 3, SW], fp16, name="out_sb",
                                       tag="out_sb")
                    # mc=2 has only 64 valid channel partitions; zero the rest
                    # so the out DMA never reads uninitialized SBUF
                    nc.gpsimd.memset(out_sb[64:128, :, 2:3, :], 0.0)

                    # ---------- input DMAs ----------
                    if ndeg:
                        sg = inp.tile([128, NSG * C], fp16, name="sg",
                                      tag="sg")
                        nc.sync.dma_start(out=sg[:], in_=sg_d[:])
                        sr = inp.tile([128, SRC], fp16, name="sr", tag="sr")
                        nc.sync.dma_start(out=sr[:], in_=sr_d[:])
                    sd_tiles, ss_tiles = [], []
                    if ndense:
                        # chunk slab streams per (dense batch, block pair)
                        off = 0
                        for di, (_, us) in enumerate(denses):
                            for b2 in range(0, NBLK, 2):
                                nsl = us[b2] + us[b2 + 1]
                                sdt = sdp.tile([128, nsl * C], fp16,
                                               tag=f"sd{b2}")
                                nc.sync.dma_start(
                                    out=sdt[:],
                                    in_=sd_d[:, off * C:(off + nsl) * C])
                                sst = ssp.tile([128, nsl * 128], fp16,
                                               tag=f"ss{b2}")
                                nc.sync.dma_start(
                                    out=sst[:],
                                    in_=ss_d[:, off * 128:(off + nsl) * 128])
                                sd_tiles += [(sdt, 0), (sdt, us[b2])]
                                ss_tiles += [(sst, 0), (sst, us[b2])]
                                off += nsl
                    if ndeg:
                        # needed only by the late expansion matmuls, and only
                        # rows 0:pi_tot — issued after the sampling streams
                        e = inp.tile([128, ndeg * SW], fp16, name="e",
                                     tag="e")
                        nc.sync.dma_start(out=e[0:pi_tot, :],
                                          in_=e_d[0:pi_tot, :])

                    # ---------- degenerate path: R ----------
                    if ndeg:
                        rsb = work.tile([128, 3, 9, pi_tot], fp16, name="rsb",
                                        tag="rsb")
                        sgo, sro = 0, 0
                        for gi, (ncols, nsl, q0, nq) in enumerate(rgroups):
                            for kc, (koff, ksz) in enumerate(MB):
                                ps_r = psA.tile([128, 512], f32,
                                                name=f"psr{gi}_{kc}",
                                                tag="psA")
                                for j in range(nsl):
                                    nc.tensor.matmul(
                                        ps_r[0:ksz, 0:ncols],
                                        sg[:, (sgo + j) * C + koff:
                                           (sgo + j) * C + koff + ksz],
                                        sr[:, sro + j * ncols:
                                           sro + (j + 1) * ncols],
                                        start=(j == 0), stop=(j == nsl - 1))
                                psq = ps_r[:, 0:ncols].rearrange(
                                    "p (t q) -> p t q", t=9)
                                nc.vector.tensor_copy(
                                    rsb[0:ksz, kc, :, q0:q0 + nq],
                                    psq[0:ksz])
                            sgo += nsl
                            sro += nsl * ncols

                    # ---------- dense sampling (first half) ----------
                    smps = []
                    if ndense:
                        for di in range(ndense):
                            smp = smpp.tile([128, 3, 2 * NBLK, 66], fp16,
                                            name=f"smp{di}", tag=f"smp{di}")
                            nc.gpsimd.memset(smp[:, :, :, 0:1], 0.0)
                            nc.gpsimd.memset(smp[:, :, :, 65:66], 0.0)
                            smps.append(smp)

                    def dense_block(di, blk):
                        _, us = denses[di]
                        smp = smps[di]
                        sdt, sdo = sd_tiles[di * NBLK + blk]
                        sst, sso = ss_tiles[di * NBLK + blk]
                        ps = psA.tile([128, 512], f32,
                                      name=f"psb{di}_{blk}", tag="psA")
                        nsl = us[blk]
                        for kc, (koff, ksz) in enumerate(MB):
                            for k in range(nsl):
                                nc.tensor.matmul(
                                    ps[0:ksz, kc * 128:(kc + 1) * 128],
                                    sdt[:, (sdo + k) * C + koff:
                                        (sdo + k) * C + koff + ksz],
                                    sst[:, (sso + k) * 128:
                                        (sso + k + 1) * 128],
                                    start=(k == 0), stop=(k == nsl - 1))
                        psv = ps[:, 0:384].rearrange("p (k r c) -> p k r c",
                                                     k=3, r=2)
                        nc.vector.tensor_copy(
                            smp[0:128, 0:2, 2 * blk:2 * blk + 2, 1:65],
                            psv[0:128, 0:2, :, :])
                        nc.vector.tensor_copy(
                            smp[0:64, 2:3, 2 * blk:2 * blk + 2, 1:65],
                            psv[0:64, 2:3, :, :])
                        if blk >= 1:
                            # +1-row shifted duplicate of the kc=2 plane in
                            # partitions 64:128, for the paired conv taps
                            nc.vector.tensor_copy(
                                smp[64:128, 2:3, 2 * blk - 1:2 * blk + 1,
                                    1:65],
                                psv[0:64, 2:3, :, :])

                    if ndense:
                        for di in range(ndense):
                            for blk in range(3):
                                dense_block(di, blk)

                    # ---------- degenerate path: T ----------
                    # bias + relu commute with the per-pixel column selection,
                    # so they are applied after the expansion matmul instead
                    if ndeg:
                        ps_t = psB.tile([128, 512], f32, name="ps_t",
                                        tag="psB")
                        k = 0
                        for kc, (koff, ksz) in enumerate(MB):
                            for tap in range(9):
                                nc.tensor.matmul(
                                    ps_t[0:pi_tot, 0:C],
                                    rsb[0:ksz, kc, tap, :],
                                    wl[0:ksz, kc * 9 * C + tap * C:
                                       kc * 9 * C + tap * C + C],
                                    start=(k == 0), stop=(k == 26))
                                k += 1
                        tsb = work.tile([128, C], fp16, name="tsb", tag="tsb")
                        nc.scalar.copy(tsb[0:pi_tot, :], ps_t[0:pi_tot, 0:C])

                    # ---------- dense sampling (second half) ----------
                    if ndense:
                        for di in range(ndense):
                            for blk in range(3, NBLK):
                                dense_block(di, blk)

                    # ---------- expansion + dense conv, interleaved per mc ----
                    # the conv matmul groups run on PE while the expansion
                    # PSUM drains complete on DVE/ACT
                    for mc, (moff, msz) in enumerate(MB):
                        if ndeg:
                            for bi in range(ndeg):
                                ps_e = psB.tile([128, 512], f32,
                                                name=f"pse{mc}_{bi}",
                                                tag="psB")
                                nc.tensor.matmul(
                                    ps_e[0:msz, :],
                                    tsb[0:pi_tot, moff:moff + msz],
                                    e[0:pi_tot, bi * SW:(bi + 1) * SW],
                                    start=True, stop=True)
                                dst = out_sb[0:msz, bi, mc, :]
                                if bi % 2 == 0:
                                    nc.vector.tensor_scalar(
                                        dst, ps_e[0:msz, :],
                                        bias_t[0:msz, mc:mc + 1], 0.0,
                                        mybir.AluOpType.add,
                                        mybir.AluOpType.max)
                                else:
                                    nc.scalar.activation(
                                        dst, ps_e[0:msz, :],
                                        mybir.ActivationFunctionType.Relu,
                                        bias=bias_t[0:msz, mc:mc + 1])
                        for di in range(ndense):
                            smp = smps[di]
                            ps_c = psB.tile([128, 512], f32,
                                            name=f"psc{di}_{mc}", tag="psB")
                            k = 0
                            N_MM = 24
                            for kc, (koff, ksz) in enumerate(MB[:2]):
                                for tap in range(9):
                                    dy, dx = tap // 3, tap % 3
                                    nc.tensor.matmul(
                                        ps_c[0:msz, :],
                                        wl[0:ksz,
                                           kc * 9 * C + tap * C + moff:
                                           kc * 9 * C + tap * C + moff + msz],
                                        smp[0:ksz, kc, 1 + dy:9 + dy,
                                            dx:dx + 64],
                                        start=(k == 0), stop=(k == N_MM - 1))
                                    k += 1
                            for dx in range(3):
                                # paired kc=2 taps dy=0 (parts 0:64) and
                                # dy=1 (parts 64:128, shifted duplicate)
                                nc.tensor.matmul(
                                    ps_c[0:msz, :],
                                    wl2[0:128, dx * C + moff:
                                        dx * C + moff + msz],
                                    smp[0:128, 2, 1:9, dx:dx + 64],
                                    start=False, stop=(k == N_MM - 1))
                                k += 1
                                # single kc=2 tap dy=2
                                nc.tensor.matmul(
                                    ps_c[0:msz, :],
                                    wl[0:64,
                                       2 * 9 * C + (6 + dx) * C + moff:
                                       2 * 9 * C + (6 + dx) * C + moff + msz],
                                    smp[0:64, 2, 3:11, dx:dx + 64],
                                    start=False, stop=(k == N_MM - 1))
                                k += 1
                            nc.scalar.activation(
                                out_sb[0:msz, ndeg + di, mc, :],
                                ps_c[0:msz, :],
                                mybir.ActivationFunctionType.Relu,
                                bias=bias_t[0:msz, mc:mc + 1])
                    for si in range(NB):
                        nc.sync.dma_start(
                            out=out_d[:, si * 3 * SW:(si + 1) * 3 * SW],
                            in_=out_sb[:, si, :, :])

            if reps == 1:
                body(0)
            else:
                # manual 3x unroll inside the hardware loop: consecutive
                # repetitions overlap through the double-buffered pools and
                # the For_i all-engine barrier is amortized over 3 reps
                U = 4
                n_loop = reps // U
                hints = (mybir.EngineType.PE, mybir.EngineType.Activation,
                         mybir.EngineType.Pool, mybir.EngineType.SP,
                         mybir.EngineType.DVE)
                with tc.For_i(0, n_loop, 1, hint_engines=hints) as it:
                    for u in range(U):
                        body(u)
                for u in range(reps - n_loop * U):
                    body(u)

    nc.finalize()
    _NC_CACHE[key] = nc
    return nc


# ---------------------------------------------------------------- interface


def make_in_maps(x, source_intrinsics, target_intrinsics, source_pose,
                 target_pose, conv_w, conv_b):
    return make_plan(x, source_intrinsics, target_intrinsics, source_pose,
                     target_pose, conv_w, conv_b)


def assemble(results, slots):
    """results: list of per-core {"out": [128, NB*3*SW]} -> [B, C, H, W]."""
    out = np.zeros((B, C, H, W), dtype=np.float32)
    NBl = len(slots)
    for r in range(NCORE):
        o = np.asarray(results[r]["out"]).reshape(128, NBl, 3, STRIP, W)
        for si, gb in enumerate(slots):
            for mc, (moff, msz) in enumerate(MB):
                out[gb, moff:moff + msz, 8 * r: 8 * r + STRIP, :] = \
                    o[0:msz, si, mc].astype(np.float32)
    return out


def kernel(x, source_intrinsics, target_intrinsics, source_pose,
           target_pose, conv_w, conv_b, _reps=1):
    from concourse.bass_utils import run_bass_kernel_spmd
    in_maps, struct, slots = make_in_maps(
        x, source_intrinsics, target_intrinsics, source_pose,
        target_pose, conv_w, conv_b)
    nc = build_program(_reps, struct)
    res = run_bass_kernel_spmd(nc, in_maps, list(range(NCORE)))
    return assemble(res.results, slots)
